# revision 14
# baseline (speedup 1.0000x reference)
"""Trainium2 Bass kernel for an FM + MLP embedding-lookup model (DeepFM-style).

Model:
    emb[b,f,:] = W2[f, cat[b,f], :] * 0.1          # [B, F, D] gather
    fm  = 0.5*((sum_f emb)^2 - sum_f emb^2)        # [B, D]
    h   = sigmoid(fm @ W_h1 + b_h1)
    h   = sigmoid(h @ W_h2 + b_h2)
    out = h @ W_out + b_out + sum_d fm + bias      # [B, 1]

Sharding: data-parallel over the batch across 8 NeuronCores; the embedding
table (F*V=2.6M rows x 64) is replicated per core, host-cast to fp16 (128B
rows, halves HBM gather traffic; end-to-end rel err ~4e-4 vs 2e-2 gate).
The gather runs as per-(chunk,field) indirect DMAs (the HW SWDGE contract:
one offset per dest partition, one contiguous run per partition).

Scaling trick: fm is quadratic in emb, so the 0.1 scale is folded as
fm = 0.005*(s^2 - ss) with s/ss computed on unscaled rows; the 0.005 lands in
the sigmoid activation's `scale` and in a pre-scaled ones-vector used to fold
sum_d(fm) into the final PSUM accumulation.
"""
import sys

for _p in ("/opt/trn_rl_repo", "/opt/pypackages"):
    if _p not in sys.path:
        sys.path.append(_p)

import numpy as np

import concourse.bacc as bacc
import concourse.mybir as mybir
import concourse.tile as tile
from concourse.bass import IndirectOffsetOnAxis

B, F, V, D = 16384, 26, 100000, 64
H1, H2 = 256, 128
NCORES = 8
BC = B // NCORES          # 2048 rows per core
P = 128                   # chunk rows == SBUF partitions
NCHUNK = BC // P          # 16

f32 = mybir.dt.float32
fp16 = mybir.dt.float16
i32 = mybir.dt.int32

_cache: dict = {}


def build_program(loop_n: int = 1):
    """Per-core SPMD program. `loop_n` > 1 wraps the body in a HW loop for
    marginal timing."""
    nc = bacc.Bacc(None, target_bir_lowering=False)

    idx_d = nc.dram_tensor("idx", [P, NCHUNK * F], i32, kind="ExternalInput")
    w2_d = nc.dram_tensor("w2", [F * V, D], fp16, kind="ExternalInput")
    wh1_d = nc.dram_tensor("wh1", [D, H1], f32, kind="ExternalInput")
    bh1_d = nc.dram_tensor("bh1", [P, H1 // P], f32, kind="ExternalInput")
    wh2_d = nc.dram_tensor("wh2", [H1, H2], f32, kind="ExternalInput")
    bh2_d = nc.dram_tensor("bh2", [H2, 1], f32, kind="ExternalInput")
    wout_d = nc.dram_tensor("wout", [H2, 1], f32, kind="ExternalInput")
    ones_d = nc.dram_tensor("ones005", [D, 1], f32, kind="ExternalInput")
    iden_d = nc.dram_tensor("iden", [P, P], f32, kind="ExternalInput")
    cb_d = nc.dram_tensor("cb", [1, 1], f32, kind="ExternalInput")
    out_d = nc.dram_tensor("out", [NCHUNK, P], f32, kind="ExternalOutput")

    with tile.TileContext(nc) as tc:
        with tc.tile_pool(name="const", bufs=1) as cp, \
             tc.tile_pool(name="work", bufs=3) as wp, \
             tc.tile_pool(name="small", bufs=2) as sp, \
             tc.tile_pool(name="psumA", bufs=2, space="PSUM") as ppa, \
             tc.tile_pool(name="psumB", bufs=1, space="PSUM") as ppb:
            idx_t = cp.tile([P, NCHUNK * F], i32)
            wh1_t = cp.tile([D, H1], f32)
            bh1_t = cp.tile([P, H1 // P], f32)
            wh2a_t = cp.tile([P, H2], f32)
            wh2b_t = cp.tile([P, H2], f32)
            bh2_t = cp.tile([H2, 1], f32)
            wout_t = cp.tile([H2, 1], f32)
            ones_t = cp.tile([D, 1], f32)
            iden_t = cp.tile([P, P], f32)
            cb_t = cp.tile([1, 1], f32)
            nc.sync.dma_start(idx_t[:], idx_d[:])
            nc.sync.dma_start(wh1_t[:], wh1_d[:])
            nc.sync.dma_start(bh1_t[:], bh1_d[:])
            nc.sync.dma_start(wh2a_t[:], wh2_d[0:P, :])
            nc.sync.dma_start(wh2b_t[:], wh2_d[P : 2 * P, :])
            nc.sync.dma_start(bh2_t[:], bh2_d[:])
            nc.sync.dma_start(wout_t[:], wout_d[:])
            nc.sync.dma_start(ones_t[:], ones_d[:])
            nc.sync.dma_start(iden_t[:], iden_d[:])
            nc.sync.dma_start(cb_t[:], cb_d[:])

            def body():
                for c in range(NCHUNK):
                    emb = wp.tile([P, F * D], fp16, tag="emb")
                    # HW indirect-DMA contract: one offset per dest partition,
                    # one contiguous block per partition -> one DMA per field.
                    for f in range(F):
                        nc.gpsimd.indirect_dma_start(
                            out=emb[:, f * D : (f + 1) * D],
                            out_offset=None,
                            in_=w2_d[:],
                            in_offset=IndirectOffsetOnAxis(
                                ap=idx_t[:, c * F + f : c * F + f + 1], axis=0
                            ),
                        )
                    sq = wp.tile([P, F * D], f32, tag="sq")
                    nc.scalar.square(sq[:], emb[:])

                    s_t = sp.tile([P, D], f32, tag="s")
                    ss_t = sp.tile([P, D], f32, tag="ss")
                    nc.vector.reduce_sum(
                        s_t[:],
                        emb[:].rearrange("p (f d) -> p d f", f=F, d=D),
                        axis=mybir.AxisListType.X,
                    )
                    nc.vector.reduce_sum(
                        ss_t[:],
                        sq[:].rearrange("p (f d) -> p d f", f=F, d=D),
                        axis=mybir.AxisListType.X,
                    )
                    # t2 = s*s - ss   (unscaled fm; the 0.005 is folded later)
                    t2 = sp.tile([P, D], f32, tag="t2")
                    nc.vector.tensor_tensor(
                        out=t2[:], in0=s_t[:], in1=s_t[:], op=mybir.AluOpType.mult
                    )
                    nc.vector.tensor_tensor(
                        out=t2[:], in0=t2[:], in1=ss_t[:], op=mybir.AluOpType.subtract
                    )

                    fmT_p = ppa.tile([D, P], f32, tag="fmT", space="PSUM")
                    nc.tensor.transpose(out=fmT_p[:], in_=t2[:], identity=iden_t[:])
                    fmT = sp.tile([D, P], f32, tag="fmT_sb")
                    nc.scalar.copy(fmT[:], fmT_p[:])

                    h1a_p = ppb.tile([P, P], f32, tag="h1a", space="PSUM")
                    h1b_p = ppb.tile([P, P], f32, tag="h1b", space="PSUM")
                    nc.tensor.matmul(
                        out=h1a_p[:], lhsT=wh1_t[:, 0:P], rhs=fmT[:], start=True, stop=True
                    )
                    nc.tensor.matmul(
                        out=h1b_p[:], lhsT=wh1_t[:, P : 2 * P], rhs=fmT[:], start=True, stop=True
                    )
                    h1a = sp.tile([P, P], f32, tag="h1a_sb")
                    h1b = sp.tile([P, P], f32, tag="h1b_sb")
                    nc.scalar.activation(
                        h1a[:], h1a_p[:], mybir.ActivationFunctionType.Sigmoid,
                        bias=bh1_t[:, 0:1], scale=0.005,
                    )
                    nc.scalar.activation(
                        h1b[:], h1b_p[:], mybir.ActivationFunctionType.Sigmoid,
                        bias=bh1_t[:, 1:2], scale=0.005,
                    )

                    h2_p = ppb.tile([P, P], f32, tag="h2", space="PSUM")
                    nc.tensor.matmul(
                        out=h2_p[:], lhsT=wh2a_t[:], rhs=h1a[:], start=True, stop=False
                    )
                    nc.tensor.matmul(
                        out=h2_p[:], lhsT=wh2b_t[:], rhs=h1b[:], start=False, stop=True
                    )
                    h2 = sp.tile([P, P], f32, tag="h2_sb")
                    nc.scalar.activation(
                        h2[:], h2_p[:], mybir.ActivationFunctionType.Sigmoid,
                        bias=bh2_t[:, 0:1],
                    )

                    fin_p = ppa.tile([1, P], f32, tag="fin", space="PSUM")
                    nc.tensor.matmul(
                        out=fin_p[:], lhsT=wout_t[:], rhs=h2[:], start=True, stop=False
                    )
                    nc.tensor.matmul(
                        out=fin_p[:], lhsT=ones_t[:], rhs=fmT[:], start=False, stop=True
                    )
                    orow = sp.tile([1, P], f32, tag="orow")
                    nc.vector.tensor_scalar_add(orow[:], fin_p[:], cb_t[0:1, 0:1])
                    nc.sync.dma_start(out_d[c : c + 1, :], orow[:])

            if loop_n > 1:
                with tc.For_i(0, loop_n) as _i:
                    body()
            else:
                body()
    nc.compile()
    return nc


def prep_inputs(cat_feat, W2, W_h1, b_h1, W_h2, b_h2, W_out, b_out, bias):
    """Host-side sharding: returns (in_maps list of 8 dicts)."""
    cat = np.asarray(cat_feat).astype(np.int64)
    flat = (np.arange(F, dtype=np.int64)[None, :] * V + cat).astype(np.int32)
    # idx per core: [P, NCHUNK*F], idx[p, c*F+f] = flat[core*BC + c*P + p, f]
    idx = flat.reshape(NCORES, NCHUNK, P, F).transpose(0, 2, 1, 3).reshape(
        NCORES, P, NCHUNK * F
    )
    w2 = np.ascontiguousarray(
        np.asarray(W2).reshape(F * V, D).astype(np.float16)
    )
    wh1 = np.ascontiguousarray(np.asarray(W_h1, dtype=np.float32))
    bh1 = np.ascontiguousarray(
        np.asarray(b_h1, dtype=np.float32).reshape(H1 // P, P).T
    )
    wh2 = np.ascontiguousarray(np.asarray(W_h2, dtype=np.float32))
    bh2 = np.asarray(b_h2, dtype=np.float32).reshape(H2, 1)
    wout = np.ascontiguousarray(np.asarray(W_out, dtype=np.float32).reshape(H2, 1))
    ones005 = np.full((D, 1), 0.005, dtype=np.float32)
    iden = np.eye(P, dtype=np.float32)
    cb = np.array(
        [[np.float32(np.asarray(b_out).reshape(-1)[0]) + np.float32(np.asarray(bias).reshape(-1)[0])]],
        dtype=np.float32,
    )
    common = {
        "w2": w2, "wh1": wh1, "bh1": bh1, "wh2": wh2, "bh2": bh2,
        "wout": wout, "ones005": ones005, "iden": iden, "cb": cb,
    }
    return [dict(common, idx=np.ascontiguousarray(idx[c])) for c in range(NCORES)]


def kernel(**inputs) -> np.ndarray:
    from concourse.bass_utils import run_bass_kernel_spmd

    if "nc" not in _cache:
        _cache["nc"] = build_program()
    nc = _cache["nc"]
    in_maps = prep_inputs(**inputs)
    res = run_bass_kernel_spmd(nc, in_maps, list(range(NCORES)))
    out = np.concatenate(
        [np.asarray(res.results[c]["out"]).reshape(BC) for c in range(NCORES)]
    )
    return out.astype(np.float32)[:, None]


# revision 15
# speedup vs baseline: 1.0393x; 1.0393x over previous
"""Trainium2 Bass kernel for an FM + MLP embedding-lookup model (DeepFM-style).

Model:
    emb[b,f,:] = W2[f, cat[b,f], :] * 0.1          # [B, F, D] gather
    fm  = 0.5*((sum_f emb)^2 - sum_f emb^2)        # [B, D]
    h   = sigmoid(fm @ W_h1 + b_h1)
    h   = sigmoid(h @ W_h2 + b_h2)
    out = h @ W_out + b_out + sum_d fm + bias      # [B, 1]

Sharding: data-parallel over the batch across 8 NeuronCores; the embedding
table (F*V=2.6M rows x 64) is replicated per core, host-cast to fp16 (128B
rows, halves HBM gather traffic; end-to-end rel err ~4e-4 vs 2e-2 gate).
The gather runs as per-(chunk,field) indirect DMAs (the HW SWDGE contract:
one offset per dest partition, one contiguous run per partition).

Scaling trick: fm is quadratic in emb, so the 0.1 scale is folded as
fm = 0.005*(s^2 - ss) with s/ss computed on unscaled rows; the 0.005 lands in
the sigmoid activation's `scale` and in a pre-scaled ones-vector used to fold
sum_d(fm) into the final PSUM accumulation.
"""
import sys

for _p in ("/opt/trn_rl_repo", "/opt/pypackages"):
    if _p not in sys.path:
        sys.path.append(_p)

import numpy as np

import concourse.bacc as bacc
import concourse.mybir as mybir
import concourse.tile as tile
from concourse.bass import IndirectOffsetOnAxis

B, F, V, D = 16384, 26, 100000, 64
H1, H2 = 256, 128
NCORES = 8
BC = B // NCORES          # 2048 rows per core
P = 128                   # chunk rows == SBUF partitions
NCHUNK = BC // P          # 16

f32 = mybir.dt.float32
fp16 = mybir.dt.float16
i32 = mybir.dt.int32

_cache: dict = {}


def build_program(loop_n: int = 1):
    """Per-core SPMD program. `loop_n` > 1 wraps the body in a HW loop for
    marginal timing."""
    nc = bacc.Bacc(None, target_bir_lowering=False)

    idx_d = nc.dram_tensor("idx", [P, NCHUNK * F], i32, kind="ExternalInput")
    w2_d = nc.dram_tensor("w2", [F * V, D], fp16, kind="ExternalInput")
    wh1_d = nc.dram_tensor("wh1", [D, H1], f32, kind="ExternalInput")
    bh1_d = nc.dram_tensor("bh1", [P, H1 // P], f32, kind="ExternalInput")
    wh2_d = nc.dram_tensor("wh2", [H1, H2], f32, kind="ExternalInput")
    bh2_d = nc.dram_tensor("bh2", [H2, 1], f32, kind="ExternalInput")
    wout_d = nc.dram_tensor("wout", [H2, 1], f32, kind="ExternalInput")
    ones_d = nc.dram_tensor("ones005", [D, 1], f32, kind="ExternalInput")
    iden_d = nc.dram_tensor("iden", [P, P], f32, kind="ExternalInput")
    cb_d = nc.dram_tensor("cb", [1, 1], f32, kind="ExternalInput")
    out_d = nc.dram_tensor("out", [NCHUNK, P], f32, kind="ExternalOutput")

    with tile.TileContext(nc) as tc:
        with tc.tile_pool(name="const", bufs=1) as cp, \
             tc.tile_pool(name="embp", bufs=NCHUNK) as ep, \
             tc.tile_pool(name="work", bufs=3) as wp, \
             tc.tile_pool(name="small", bufs=2) as sp, \
             tc.tile_pool(name="psumA", bufs=2, space="PSUM") as ppa, \
             tc.tile_pool(name="psumB", bufs=1, space="PSUM") as ppb:
            idx_t = cp.tile([P, NCHUNK * F], i32)
            wh1_t = cp.tile([D, H1], f32)
            bh1_t = cp.tile([P, H1 // P], f32)
            wh2a_t = cp.tile([P, H2], f32)
            wh2b_t = cp.tile([P, H2], f32)
            bh2_t = cp.tile([H2, 1], f32)
            wout_t = cp.tile([H2, 1], f32)
            ones_t = cp.tile([D, 1], f32)
            iden_t = cp.tile([P, P], f32)
            cb_t = cp.tile([1, 1], f32)
            nc.sync.dma_start(idx_t[:], idx_d[:])
            nc.sync.dma_start(wh1_t[:], wh1_d[:])
            nc.sync.dma_start(bh1_t[:], bh1_d[:])
            nc.sync.dma_start(wh2a_t[:], wh2_d[0:P, :])
            nc.sync.dma_start(wh2b_t[:], wh2_d[P : 2 * P, :])
            nc.sync.dma_start(bh2_t[:], bh2_d[:])
            nc.sync.dma_start(wout_t[:], wout_d[:])
            nc.sync.dma_start(ones_t[:], ones_d[:])
            nc.sync.dma_start(iden_t[:], iden_d[:])
            nc.sync.dma_start(cb_t[:], cb_d[:])

            def body():
                for c in range(NCHUNK):
                    emb = ep.tile([P, F * D], fp16, tag="emb")
                    # HW indirect-DMA contract: one offset per dest partition,
                    # one contiguous block per partition -> one DMA per field.
                    for f in range(F):
                        nc.gpsimd.indirect_dma_start(
                            out=emb[:, f * D : (f + 1) * D],
                            out_offset=None,
                            in_=w2_d[:],
                            in_offset=IndirectOffsetOnAxis(
                                ap=idx_t[:, c * F + f : c * F + f + 1], axis=0
                            ),
                        )
                    sq = wp.tile([P, F * D], f32, tag="sq")
                    nc.scalar.square(sq[:], emb[:])

                    s_t = sp.tile([P, D], f32, tag="s")
                    ss_t = sp.tile([P, D], f32, tag="ss")
                    nc.vector.reduce_sum(
                        s_t[:],
                        emb[:].rearrange("p (f d) -> p d f", f=F, d=D),
                        axis=mybir.AxisListType.X,
                    )
                    nc.vector.reduce_sum(
                        ss_t[:],
                        sq[:].rearrange("p (f d) -> p d f", f=F, d=D),
                        axis=mybir.AxisListType.X,
                    )
                    # t2 = s*s - ss   (unscaled fm; the 0.005 is folded later)
                    t2 = sp.tile([P, D], f32, tag="t2")
                    nc.vector.tensor_tensor(
                        out=t2[:], in0=s_t[:], in1=s_t[:], op=mybir.AluOpType.mult
                    )
                    nc.vector.tensor_tensor(
                        out=t2[:], in0=t2[:], in1=ss_t[:], op=mybir.AluOpType.subtract
                    )

                    fmT_p = ppa.tile([D, P], f32, tag="fmT", space="PSUM")
                    nc.tensor.transpose(out=fmT_p[:], in_=t2[:], identity=iden_t[:])
                    fmT = sp.tile([D, P], f32, tag="fmT_sb")
                    nc.scalar.copy(fmT[:], fmT_p[:])

                    h1a_p = ppb.tile([P, P], f32, tag="h1a", space="PSUM")
                    h1b_p = ppb.tile([P, P], f32, tag="h1b", space="PSUM")
                    nc.tensor.matmul(
                        out=h1a_p[:], lhsT=wh1_t[:, 0:P], rhs=fmT[:], start=True, stop=True
                    )
                    nc.tensor.matmul(
                        out=h1b_p[:], lhsT=wh1_t[:, P : 2 * P], rhs=fmT[:], start=True, stop=True
                    )
                    h1a = sp.tile([P, P], f32, tag="h1a_sb")
                    h1b = sp.tile([P, P], f32, tag="h1b_sb")
                    nc.scalar.activation(
                        h1a[:], h1a_p[:], mybir.ActivationFunctionType.Sigmoid,
                        bias=bh1_t[:, 0:1], scale=0.005,
                    )
                    nc.scalar.activation(
                        h1b[:], h1b_p[:], mybir.ActivationFunctionType.Sigmoid,
                        bias=bh1_t[:, 1:2], scale=0.005,
                    )

                    h2_p = ppb.tile([P, P], f32, tag="h2", space="PSUM")
                    nc.tensor.matmul(
                        out=h2_p[:], lhsT=wh2a_t[:], rhs=h1a[:], start=True, stop=False
                    )
                    nc.tensor.matmul(
                        out=h2_p[:], lhsT=wh2b_t[:], rhs=h1b[:], start=False, stop=True
                    )
                    h2 = sp.tile([P, P], f32, tag="h2_sb")
                    nc.scalar.activation(
                        h2[:], h2_p[:], mybir.ActivationFunctionType.Sigmoid,
                        bias=bh2_t[:, 0:1],
                    )

                    fin_p = ppa.tile([1, P], f32, tag="fin", space="PSUM")
                    nc.tensor.matmul(
                        out=fin_p[:], lhsT=wout_t[:], rhs=h2[:], start=True, stop=False
                    )
                    nc.tensor.matmul(
                        out=fin_p[:], lhsT=ones_t[:], rhs=fmT[:], start=False, stop=True
                    )
                    orow = sp.tile([1, P], f32, tag="orow")
                    nc.vector.tensor_scalar_add(orow[:], fin_p[:], cb_t[0:1, 0:1])
                    nc.sync.dma_start(out_d[c : c + 1, :], orow[:])

            if loop_n > 1:
                with tc.For_i(0, loop_n) as _i:
                    body()
            else:
                body()
    nc.compile()
    return nc


def prep_inputs(cat_feat, W2, W_h1, b_h1, W_h2, b_h2, W_out, b_out, bias):
    """Host-side sharding: returns (in_maps list of 8 dicts)."""
    cat = np.asarray(cat_feat).astype(np.int64)
    flat = (np.arange(F, dtype=np.int64)[None, :] * V + cat).astype(np.int32)
    # idx per core: [P, NCHUNK*F], idx[p, c*F+f] = flat[core*BC + c*P + p, f]
    idx = flat.reshape(NCORES, NCHUNK, P, F).transpose(0, 2, 1, 3).reshape(
        NCORES, P, NCHUNK * F
    )
    w2 = np.ascontiguousarray(
        np.asarray(W2).reshape(F * V, D).astype(np.float16)
    )
    wh1 = np.ascontiguousarray(np.asarray(W_h1, dtype=np.float32))
    bh1 = np.ascontiguousarray(
        np.asarray(b_h1, dtype=np.float32).reshape(H1 // P, P).T
    )
    wh2 = np.ascontiguousarray(np.asarray(W_h2, dtype=np.float32))
    bh2 = np.asarray(b_h2, dtype=np.float32).reshape(H2, 1)
    wout = np.ascontiguousarray(np.asarray(W_out, dtype=np.float32).reshape(H2, 1))
    ones005 = np.full((D, 1), 0.005, dtype=np.float32)
    iden = np.eye(P, dtype=np.float32)
    cb = np.array(
        [[np.float32(np.asarray(b_out).reshape(-1)[0]) + np.float32(np.asarray(bias).reshape(-1)[0])]],
        dtype=np.float32,
    )
    common = {
        "w2": w2, "wh1": wh1, "bh1": bh1, "wh2": wh2, "bh2": bh2,
        "wout": wout, "ones005": ones005, "iden": iden, "cb": cb,
    }
    return [dict(common, idx=np.ascontiguousarray(idx[c])) for c in range(NCORES)]


def kernel(**inputs) -> np.ndarray:
    from concourse.bass_utils import run_bass_kernel_spmd

    if "nc" not in _cache:
        _cache["nc"] = build_program()
    nc = _cache["nc"]
    in_maps = prep_inputs(**inputs)
    res = run_bass_kernel_spmd(nc, in_maps, list(range(NCORES)))
    out = np.concatenate(
        [np.asarray(res.results[c]["out"]).reshape(BC) for c in range(NCORES)]
    )
    return out.astype(np.float32)[:, None]
